# revision 10
# baseline (speedup 1.0000x reference)
"""Trainium2 Bass kernel for nn_HadamardProj.

The reference's "FWHT" butterfly pairs the SAME adjacent elements every
step: one step T satisfies T^2 = 2*I, so log2(1024)=10 steps give
T^10 = 32*I, exactly cancelled by the final d**-0.5 = 1/32 scaling.
Each fwht() is therefore the identity (up to fp rounding), and the whole
model collapses to an elementwise multiply:

    y = x * (s0 * s1 * s2 * s3 * s4)        # broadcast along D

which is a pure memory-bound streaming kernel: read 64 MB, write 64 MB.
We shard the 16384 rows across 8 NeuronCores (2048 rows = 8 MB/core),
view each shard as (128 partitions, 16384 free), and stream 512 KiB
tiles through SBUF with an in-place vector multiply against the
combined scale vector (broadcast to 128 partitions on-chip by GpSimd).

The primary build is raw Bass (no TileContext): the DMA bus is saturated
end to end, so the only compressible time is the prologue/epilogue.
Per-core schedule:

  SP   : load0 | sem_clear(S) | loads 1..15 | drain(wait S>=256)
  Act  : sem_clear(Dp) | store_i (wait Dp>=i+1, inc S by 16)
  DVE  : sem_clear(L0..15, B) | wait B | mults (tile i waits L_i>=16)
  Pool : sem_clear(SW) | swdge-load s_row (inc SW 16) | partition_broadcast
         (wait SW>=16, inc B)

Every engine clears exactly the semaphores it waits on as its first
instructions; all increments of those sems arrive >2.9 us later (gated
by DMA transfers), so the clears are ordered ahead of every update
without any barrier and the NEFF stays re-runnable with dirty sems.
This removes the module-entry barrier and the TileContext exit
(drain + barrier + sem-release + barrier) from the critical path:
first byte on the DMA bus at t=1300 ns (SEQ decode + HWDGE gen + DGE
delay), gap-free bus, last event at bus-end + the mandatory 900 ns
DMA-completion-semaphore propagation of the final store.
"""

import numpy as np
from contextlib import ExitStack

import concourse.bacc as bacc
import concourse.bass as bass
import concourse.tile as tile
import concourse.mybir as mybir
from concourse.mybir import AluOpType
from concourse.bass_utils import run_bass_kernel_spmd

N_CORES = 8
B, S, D = 4, 4096, 1024
ROWS = B * S                        # 16384
ROWS_PER_CORE = ROWS // N_CORES     # 2048
P = 128
FREE = ROWS_PER_CORE * D // P       # 16384 f32 per partition (64 KB)
CHUNK = 1024                        # free-dim chunk -> (128, 1024) = 512 KiB tiles
N_CHUNKS = FREE // CHUNK            # 16
D_PER_CHUNK = CHUNK // D            # 1 multiply of (128, D) per chunk
FB_CHUNK = 2048                     # fallback builds keep their proven 1 MiB tiling
FB_N_CHUNKS = FREE // FB_CHUNK      # 8
FB_D_PER_CHUNK = FB_CHUNK // D      # 2
BUFS = 8                            # tile-pool slots for the fallback build

_nc_cache = None          # (nc, scale_mode) once built
FORCE_MODE = None         # test hook: None | "fast" | "row" | "full"


def _build_nc_fast():
    # Raw Bass, no TileContext. Bass.__init__ ends with an all-engine
    # barrier after the const-AP memsets; we use none of the const APs and
    # every engine self-clears the sems it waits on, so the barrier would
    # only delay the first load's descriptor generation by ~600 ns.
    # Suppress it for the constructor.
    orig_barrier = bass.Bass.all_engine_barrier
    bass.Bass.all_engine_barrier = lambda self, **kw: None
    try:
        nc = bacc.Bacc("TRN2", target_bir_lowering=False, debug=False)
    finally:
        bass.Bass.all_engine_barrier = orig_barrier

    x_d = nc.dram_tensor("x", [P, FREE], mybir.dt.float32, kind="ExternalInput").ap()
    s_d = nc.dram_tensor("scale", [1, D], mybir.dt.float32, kind="ExternalInput").ap()
    y_d = nc.dram_tensor("y", [P, FREE], mybir.dt.float32, kind="ExternalOutput").ap()

    with ExitStack() as ctx:
        L = [ctx.enter_context(nc.semaphore(f"Ld{i}")) for i in range(N_CHUNKS)]
        Bc = ctx.enter_context(nc.semaphore("Bcast"))
        SW = ctx.enter_context(nc.semaphore("SRow"))
        Dp = ctx.enter_context(nc.semaphore("Dprog"))
        St = ctx.enter_context(nc.semaphore("Stores"))

        xt = [
            ctx.enter_context(nc.sbuf_tensor(f"xt{i}", [P, CHUNK], mybir.dt.float32))
            for i in range(N_CHUNKS)
        ]
        s_row = ctx.enter_context(nc.sbuf_tensor("s_row", [1, D], mybir.dt.float32))
        s_b = ctx.enter_context(nc.sbuf_tensor("s_b", [P, D], mybir.dt.float32))

        # SP: the first load issues immediately; its DGE latency is the
        # only thing between t=0 and the first byte on the DMA bus.
        nc.sync.dma_start(xt[0][:, :], x_d[:, 0:CHUNK]).then_inc(L[0], 16)
        nc.sync.sem_clear(St)
        for i in range(1, N_CHUNKS):
            nc.sync.dma_start(
                xt[i][:, :], x_d[:, i * CHUNK:(i + 1) * CHUNK]
            ).then_inc(L[i], 16)
        # Gate NEFF completion on every store having fully landed in DRAM:
        # 8 stores x 16 DMA-engine increments each. A wait-carrying Drain
        # (the same construct TileContext's epilogue uses) completes the
        # moment the semaphore arrives, unlike an EventSemaphore which pays
        # its sequencer exec slot after the wait resolves.
        nc.sync.drain(fusable=False)._wait_ge(St, 16 * N_CHUNKS)

        # Activation: stores, gated on DVE progress (Dp is incremented only
        # by DVE in program order, so Dp >= i+1 means tiles 0..i are done).
        # Every DMA must carry a completion semaphore (walrus codegen
        # SIGABRTs otherwise), so the final store's 900 ns sem propagation
        # is an irreducible part of the tail.
        nc.scalar.sem_clear(Dp)
        for i in range(N_CHUNKS):
            st = nc.scalar.dma_start(y_d[:, i * CHUNK:(i + 1) * CHUNK], xt[i][:, :])
            st._wait_ge(Dp, i + 1)
            st.then_inc(St, 16)

        # DVE: in-place multiplies.
        for i in range(N_CHUNKS):
            nc.vector.sem_clear(L[i])
        nc.vector.sem_clear(Bc)
        nc.vector.wait_ge(Bc, 1)
        for i in range(N_CHUNKS):
            for k in range(D_PER_CHUNK):
                tt = nc.vector.tensor_tensor(
                    xt[i][:, k * D:(k + 1) * D],
                    xt[i][:, k * D:(k + 1) * D],
                    s_b[:, :],
                    AluOpType.mult,
                )
                if k == 0:
                    tt._wait_ge(L[i], 16)
                if k == D_PER_CHUNK - 1:
                    tt.then_inc(Dp, 1)

        # Pool/GpSimd: combined-scale row in via SWDGE, broadcast across
        # partitions so the 512 KB replication never touches the DMA bus.
        nc.gpsimd.sem_clear(SW)
        nc.gpsimd.dma_start(s_row[:, :], s_d[:, :]).then_inc(SW, 16)
        bc = nc.gpsimd.partition_broadcast(s_b[:, :], s_row[:, :])
        bc._wait_ge(SW, 16)
        bc.then_inc(Bc, 1)

    nc.compile()
    return nc


def _build_nc_fallback():
    # Conservative variant: no gpsimd ucode ops. The combined scale arrives
    # pre-broadcast from the host as a (128, D) input and is DMA'd once
    # (512 KB, ~1.4 us of bus time). ~3% slower than the tile path but
    # uses only plain DMACopy + tensor_tensor.
    nc = bacc.Bacc("TRN2", target_bir_lowering=False, debug=False)
    x_d = nc.dram_tensor("x", [P, FREE], mybir.dt.float32, kind="ExternalInput").ap()
    s_d = nc.dram_tensor("scale", [P, D], mybir.dt.float32, kind="ExternalInput").ap()
    y_d = nc.dram_tensor("y", [P, FREE], mybir.dt.float32, kind="ExternalOutput").ap()

    with tile.TileContext(nc) as tc:
        with ExitStack() as ctx:
            const_pool = ctx.enter_context(tc.tile_pool(name="const", bufs=1))
            xpool = ctx.enter_context(tc.tile_pool(name="x", bufs=BUFS))

            s_b = const_pool.tile([P, D], mybir.dt.float32)
            nc.scalar.dma_start(s_b[:], s_d[:])

            for i in range(FB_N_CHUNKS):
                t = xpool.tile([P, FB_CHUNK], mybir.dt.float32)
                nc.sync.dma_start(t[:], x_d[:, i * FB_CHUNK:(i + 1) * FB_CHUNK])
                for k in range(FB_D_PER_CHUNK):
                    nc.vector.tensor_tensor(
                        t[:, k * D:(k + 1) * D],
                        t[:, k * D:(k + 1) * D],
                        s_b[:],
                        AluOpType.mult,
                    )
                nc.scalar.dma_start(y_d[:, i * FB_CHUNK:(i + 1) * FB_CHUNK], t[:])

    nc.compile()
    return nc


def _build_nc_tile():
    # Previous primary: TileContext + gpsimd broadcast. Kept as the first
    # fallback should the raw-bass build misbehave at run time.
    nc = bacc.Bacc("TRN2", target_bir_lowering=False, debug=False)
    x_d = nc.dram_tensor("x", [P, FREE], mybir.dt.float32, kind="ExternalInput").ap()
    s_d = nc.dram_tensor("scale", [1, D], mybir.dt.float32, kind="ExternalInput").ap()
    y_d = nc.dram_tensor("y", [P, FREE], mybir.dt.float32, kind="ExternalOutput").ap()

    with tile.TileContext(nc) as tc:
        with ExitStack() as ctx:
            const_pool = ctx.enter_context(tc.tile_pool(name="const", bufs=1))
            xpool = ctx.enter_context(tc.tile_pool(name="x", bufs=BUFS))

            s_row = const_pool.tile([1, D], mybir.dt.float32)
            nc.gpsimd.dma_start(s_row[:], s_d[:])
            s_b = const_pool.tile([P, D], mybir.dt.float32)
            nc.gpsimd.partition_broadcast(s_b[:], s_row[:])

            for i in range(FB_N_CHUNKS):
                t = xpool.tile([P, FB_CHUNK], mybir.dt.float32)
                nc.sync.dma_start(t[:], x_d[:, i * FB_CHUNK:(i + 1) * FB_CHUNK])
                for k in range(FB_D_PER_CHUNK):
                    nc.vector.tensor_tensor(
                        t[:, k * D:(k + 1) * D],
                        t[:, k * D:(k + 1) * D],
                        s_b[:],
                        AluOpType.mult,
                    )
                nc.scalar.dma_start(y_d[:, i * FB_CHUNK:(i + 1) * FB_CHUNK], t[:])

    nc.compile()
    return nc


_BUILDERS = {
    "fast": (_build_nc_fast, "row"),
    "row": (_build_nc_tile, "row"),
    "full": (_build_nc_fallback, "full"),
}
_MODE_ORDER = ["fast", "row", "full"]


def _get_nc(mode=None):
    global _nc_cache
    if mode is None and _nc_cache is not None:
        return _nc_cache
    modes = [mode or FORCE_MODE] if (mode or FORCE_MODE) else _MODE_ORDER
    last_err = None
    for m in modes:
        builder, scale_mode = _BUILDERS[m]
        try:
            _nc_cache = (builder(), scale_mode, m)
            return _nc_cache
        except Exception as e:  # pragma: no cover - build-env dependent
            last_err = e
    raise last_err


def _make_in_maps(x, scales, scale_mode):
    x = np.ascontiguousarray(np.asarray(x, dtype=np.float32))
    scales = np.asarray(scales, dtype=np.float32)
    comb = (scales[0] * scales[1] * scales[2] * scales[3] * scales[4]).astype(
        np.float32
    )
    if scale_mode == "row":
        s_b = np.ascontiguousarray(comb.reshape(1, D))
    else:
        s_b = np.ascontiguousarray(np.broadcast_to(comb.reshape(1, D), (P, D)))
    xf = x.reshape(ROWS, D)
    in_maps = []
    for c in range(N_CORES):
        shard = np.ascontiguousarray(
            xf[c * ROWS_PER_CORE:(c + 1) * ROWS_PER_CORE]
        ).reshape(P, FREE)
        in_maps.append({"x": shard, "scale": s_b})
    return in_maps


def _gather(results):
    out = np.empty((ROWS, D), np.float32)
    for c in range(N_CORES):
        out[c * ROWS_PER_CORE:(c + 1) * ROWS_PER_CORE] = results[c]["y"].reshape(
            ROWS_PER_CORE, D
        )
    return out.reshape(B, S, D)


def kernel(x, scales, **run_kwargs):
    global _nc_cache
    nc, scale_mode, mode = _get_nc()
    try:
        in_maps = _make_in_maps(x, scales, scale_mode)
        res = run_bass_kernel_spmd(
            nc, in_maps, core_ids=list(range(N_CORES)), **run_kwargs
        )
    except Exception:
        # Primary path failed at run time in this environment — fall back
        # down the build chain and retry.
        idx = _MODE_ORDER.index(mode)
        if idx + 1 >= len(_MODE_ORDER):
            raise
        res = None
        for m in _MODE_ORDER[idx + 1:]:
            try:
                _nc_cache = None
                nc, scale_mode, mode = _get_nc(m)
                in_maps = _make_in_maps(x, scales, scale_mode)
                res = run_bass_kernel_spmd(
                    nc, in_maps, core_ids=list(range(N_CORES)), **run_kwargs
                )
                break
            except Exception:
                continue
        if res is None:
            raise
    out = _gather(res.results)
    if run_kwargs:
        return out, res
    return out
